# revision 10
# baseline (speedup 1.0000x reference)
"""CohortAwareBlock Trainium2 kernel.

Data-parallel over batch B=8 across 8 NeuronCores (one sample per core).
The cohort routing (gather of cohort_q_w by per-sample cohort id) happens on
the host while building each core's weight tensors, so the device kernel is a
plain attention block:

  per core (sample b):
    qT = (Wq_b * scale) @ x^T + bq_b*scale       [1024, N]   (e on partitions)
    kT = Wk @ x^T + bk                           [1024, N]
    v  = x @ Wv^T + bv                           [N, 1024]
    per head h (16 heads, hd=64):
      scoresT = kT_h^T-contract: [k, q] tiles via matmul(lhsT=kT_h chunk, rhs=qT_h)
      expT    = exp(scoresT)                      (no max-subtraction: |scores| < ~5)
      out2T   = [v_h | 1]^T-matmul: rows 0:64 = unnormalized attn-out^T,
                row 64 = softmax denominator (ones-column trick)
      store unnormalized attn-out^T and 1/denominator
    proj: normalize attn-out^T rows by per-(head, n) reciprocal (broadcast
          loaded via partition-replicating DMA), then out = attnT^T @ projT + bp
"""

import numpy as np

import concourse.bass as bass
import concourse.bacc as bacc
import concourse.mybir as mybir
import concourse.tile as tile
from concourse.bass_utils import run_bass_kernel_spmd

P = 128
N = 1024            # sequence length
D = 1024            # model dim
H = 16              # heads
HD = 64             # head dim
NH = 2              # 512-wide halves of N
DC = D // P         # 8 contraction chunks
SCALE = HD ** -0.5
NCORES = 8

F32 = mybir.dt.float32

# Matmul-input dtype knob: mybir.dt.float32 (exact, 4 cyc/row),
# mybir.dt.bfloat16 (1 cyc/row), mybir.dt.float32r (fp32 data, fast path).
MM_DT = mybir.dt.float32r


def _np_dt(dt):
    return mybir.dt.np(dt)


def build_nc(mm_dt=MM_DT):
    nc = bacc.Bacc(
        "TRN2",
        target_bir_lowering=False,
        debug=False,
        num_devices=NCORES,
    )

    # ---- external I/O (per-core shards, host-prepped layouts) ----
    xT = nc.dram_tensor("xT", [D, N], mm_dt, kind="ExternalInput")       # x^T
    wqk = nc.dram_tensor("wqk", [D, 2 * D], mm_dt, kind="ExternalInput")  # [D, e] cols: q(1024, scaled) then k(1024)
    bqk = nc.dram_tensor("bqk", [2 * D], F32, kind="ExternalInput")
    wv = nc.dram_tensor("wv", [D, D], mm_dt, kind="ExternalInput")        # [D, e_v]
    bv = nc.dram_tensor("bv", [D], F32, kind="ExternalInput")
    wp = nc.dram_tensor("wp", [D, D], mm_dt, kind="ExternalInput")        # proj_w^T: [e, f]
    bp = nc.dram_tensor("bp", [D], F32, kind="ExternalInput")
    out = nc.dram_tensor("out", [N, D], F32, kind="ExternalOutput")

    with tile.TileContext(nc) as tc:
        kernel_body(tc, xT, wqk, bqk, wv, bv, wp, bp, out, mm_dt)
    nc.compile()
    return nc


def kernel_body(tc, xT, wqk, bqk, wv, bv, wp, bp, out, mm_dt):
    nc = tc.nc
    EXP = mybir.ActivationFunctionType.Exp

    from contextlib import ExitStack

    with ExitStack() as ctx:
        resident = ctx.enter_context(tc.tile_pool(name="resident", bufs=1))
        dram = ctx.enter_context(tc.tile_pool(name="dram", bufs=1, space="DRAM"))
        psum = ctx.enter_context(tc.tile_pool(name="psum", bufs=4, space="PSUM"))
        psum_o2 = ctx.enter_context(tc.tile_pool(name="psum_o2", bufs=2, space="PSUM"))

        # ---- resident tiles ----
        xT_sb = resident.tile([P, DC, N], mm_dt)
        nc.sync.dma_start(xT_sb[:], xT[:].rearrange("(dc p) n -> p dc n", p=P))

        bqk_sb = resident.tile([HD, 32], F32)  # col t = bias for e-range t*64
        nc.sync.dma_start(bqk_sb[:], bqk[:].rearrange("(t p) -> p t", p=HD))

        bv_rep = resident.tile([P, D], F32)
        nc.sync.dma_start(bv_rep[:], bv[None, :].to_broadcast([P, D]))
        bp_rep = resident.tile([P, D], F32)
        nc.sync.dma_start(bp_rep[:], bp[None, :].to_broadcast([P, D]))

        wp_sb = resident.tile([P, DC, D], mm_dt)  # proj weights [e-part, e-chunk, f]
        nc.sync.dma_start(wp_sb[:], wp[:].rearrange("(co p) f -> p co f", p=P))

        # ---- DRAM scratch ----
        qkT_d = dram.tile([2 * D, N], mm_dt)   # rows: q heads (0:1024), k heads (1024:2048)
        v_d = dram.tile([N, D], mm_dt)
        attnT_d = dram.tile([D, N], mm_dt)     # unnormalized attn-out^T
        recip_d = dram.tile([H, N], F32)       # per-(head, n) 1/denominator

        # ================= Phase A: qT / kT generation =================
        with tc.tile_pool(name="wqk_pool", bufs=3) as wqk_pool, tc.tile_pool(
            name="qk_ev", bufs=3
        ) as qk_ev:
            wqk_r = wqk[:].rearrange("(dc p) e -> p dc e", p=P)
            for t in range(32):  # 64-wide e-tiles over q(16) then k(16)
                wc = wqk_pool.tile([P, DC, HD], mm_dt, tag="wc")
                nc.sync.dma_start(wc[:], wqk_r[:, :, t * HD : (t + 1) * HD])
                for nh in range(NH):
                    ps = psum.tile([P, 512], F32, tag="ps")
                    for dc in range(DC):
                        nc.tensor.matmul(
                            ps[:HD, :],
                            lhsT=wc[:, dc, :],
                            rhs=xT_sb[:, dc, nh * 512 : (nh + 1) * 512],
                            start=(dc == 0),
                            stop=(dc == DC - 1),
                        )
                    ev = qk_ev.tile([HD, 512], mm_dt, tag="ev")
                    nc.vector.tensor_scalar_add(ev[:], ps[:HD, :], bqk_sb[:, t : t + 1])
                    nc.sync.dma_start(
                        qkT_d[t * HD : (t + 1) * HD, nh * 512 : (nh + 1) * 512], ev[:]
                    )

        # ================= Phase B: v generation =================
        with tc.tile_pool(name="wv_pool", bufs=1) as wv_pool, tc.tile_pool(
            name="v_ev", bufs=3
        ) as v_ev:
            wv_sb = wv_pool.tile([P, DC, D], mm_dt)
            nc.sync.dma_start(wv_sb[:], wv[:].rearrange("(dc p) e -> p dc e", p=P))
            for nt in range(8):
                for eh in range(NH):
                    ps = psum.tile([P, 512], F32, tag="ps")
                    for dc in range(DC):
                        nc.tensor.matmul(
                            ps[:],
                            lhsT=xT_sb[:, dc, nt * P : (nt + 1) * P],
                            rhs=wv_sb[:, dc, eh * 512 : (eh + 1) * 512],
                            start=(dc == 0),
                            stop=(dc == DC - 1),
                        )
                    ev = v_ev.tile([P, 512], mm_dt, tag="vev")
                    nc.vector.tensor_add(
                        ev[:], ps[:], bv_rep[:, eh * 512 : (eh + 1) * 512]
                    )
                    nc.sync.dma_start(
                        v_d[nt * P : (nt + 1) * P, eh * 512 : (eh + 1) * 512], ev[:]
                    )

        # ================= Phase C: attention per head =================
        v_d_r = v_d[:].rearrange("(kc p) e -> p kc e", p=P)
        with tc.tile_pool(name="qk_h_pool", bufs=2) as qk_h_pool, tc.tile_pool(
            name="v_aug_pool", bufs=2
        ) as v_aug_pool, tc.tile_pool(name="exp_pool", bufs=2) as exp_pool, tc.tile_pool(
            name="att_ev", bufs=3
        ) as att_ev:
            for h in range(H):
                qk_h = qk_h_pool.tile([P, 2, N], mm_dt, tag="qkh")
                nc.vector.memset(qk_h[HD:P, :, :].bitcast(F32), 0.0)
                nc.sync.dma_start(qk_h[:HD, 0, :], qkT_d[h * HD : (h + 1) * HD, :])
                nc.sync.dma_start(
                    qk_h[:HD, 1, :], qkT_d[D + h * HD : D + (h + 1) * HD, :]
                )
                v_aug = v_aug_pool.tile([P, 8, HD + 1], mm_dt, tag="vaug")
                nc.sync.dma_start(
                    v_aug[:, :, 0:HD], v_d_r[:, :, h * HD : (h + 1) * HD]
                )
                nc.vector.memset(v_aug[:, :, HD : HD + 1].bitcast(F32), 1.0)

                for qh in range(NH):
                    exp_sb = exp_pool.tile([P, 8, 512], mm_dt, tag="exps")
                    for kt in range(8):
                        ps_sc = psum.tile([P, 512], F32, tag="ps")
                        nc.tensor.matmul(
                            ps_sc[:],
                            lhsT=qk_h[:, 1, kt * P : (kt + 1) * P],
                            rhs=qk_h[:, 0, qh * 512 : (qh + 1) * 512],
                            start=True,
                            stop=True,
                        )
                        nc.scalar.activation(exp_sb[:, kt, :], ps_sc[:], EXP)
                    ps_o2 = psum_o2.tile([HD + 1, 512], F32, tag="o2")
                    for kt in range(8):
                        nc.tensor.matmul(
                            ps_o2[:],
                            lhsT=v_aug[:, kt, :],
                            rhs=exp_sb[:, kt, :],
                            start=(kt == 0),
                            stop=(kt == 7),
                        )
                    att = att_ev.tile([HD, 512], mm_dt, tag="att")
                    nc.vector.tensor_copy(att[:], ps_o2[:HD, :])
                    rec = att_ev.tile([HD + 1, 512], F32, tag="rec")
                    nc.vector.reciprocal(rec[HD : HD + 1, :], ps_o2[HD : HD + 1, :])
                    nc.sync.dma_start(
                        attnT_d[h * HD : (h + 1) * HD, qh * 512 : (qh + 1) * 512],
                        att[:],
                    )
                    nc.sync.dma_start(
                        recip_d[h : h + 1, qh * 512 : (qh + 1) * 512],
                        rec[HD : HD + 1, :],
                    )

        # ================= Phase D: projection =================
        with tc.tile_pool(name="norm_pool", bufs=8) as norm_pool, tc.tile_pool(
            name="proj_ld", bufs=2
        ) as proj_ld, tc.tile_pool(name="out_ev", bufs=3) as out_ev:
            nm_tiles = []
            for co in range(DC):
                at = proj_ld.tile([P, N], mm_dt, tag="at")
                nc.sync.dma_start(at[:], attnT_d[co * P : (co + 1) * P, :])
                rc = proj_ld.tile([P, N], F32, tag="rc")
                nc.sync.dma_start(
                    rc[0:HD, :],
                    recip_d[2 * co : 2 * co + 1, :].to_broadcast([HD, N]),
                )
                nc.sync.dma_start(
                    rc[HD:P, :],
                    recip_d[2 * co + 1 : 2 * co + 2, :].to_broadcast([HD, N]),
                )
                nm = norm_pool.tile([P, N], mm_dt, tag="nm")
                nc.vector.tensor_mul(nm[:], at[:], rc[:])
                nm_tiles.append(nm)

            for nt in range(8):
                for fh in range(NH):
                    ps = psum.tile([P, 512], F32, tag="ps")
                    for co in range(DC):
                        nc.tensor.matmul(
                            ps[:],
                            lhsT=nm_tiles[co][:, nt * P : (nt + 1) * P],
                            rhs=wp_sb[:, co, fh * 512 : (fh + 1) * 512],
                            start=(co == 0),
                            stop=(co == DC - 1),
                        )
                    ev = out_ev.tile([P, 512], F32, tag="oev")
                    nc.vector.tensor_add(
                        ev[:], ps[:], bp_rep[:, fh * 512 : (fh + 1) * 512]
                    )
                    nc.sync.dma_start(
                        out[nt * P : (nt + 1) * P, fh * 512 : (fh + 1) * 512], ev[:]
                    )


def make_in_maps(x, c, kv_w, kv_b, shared_q_w, shared_q_b, cohort_q_w, cohort_q_b,
                 proj_w, proj_b, mm_dt=MM_DT):
    np_dt = _np_dt(mm_dt)
    f32 = np.float32
    x = np.asarray(x, dtype=f32)
    c = np.asarray(c).astype(np.int64)
    kv_w = np.asarray(kv_w, dtype=f32)
    kv_b = np.asarray(kv_b, dtype=f32)
    shared_q_w = np.asarray(shared_q_w, dtype=f32)
    shared_q_b = np.asarray(shared_q_b, dtype=f32)
    cohort_q_w = np.asarray(cohort_q_w, dtype=f32)
    cohort_q_b = np.asarray(cohort_q_b, dtype=f32)
    proj_w = np.asarray(proj_w, dtype=f32)
    proj_b = np.asarray(proj_b, dtype=f32)

    wk = kv_w[:D]       # [1024, D]
    wv_ = kv_w[D:]      # [1024, D]
    bk = kv_b[:D]
    bv_ = kv_b[D:]

    wvT = np.ascontiguousarray(wv_.T).astype(np_dt)          # [D, e_v]
    wpT = np.ascontiguousarray(proj_w.T).astype(np_dt)       # [e, f]
    bp_ = np.ascontiguousarray(proj_b)

    in_maps = []
    for b in range(x.shape[0]):
        wq = np.concatenate([shared_q_w, cohort_q_w[c[b]]], axis=0) * SCALE
        bq = np.concatenate([shared_q_b, cohort_q_b[c[b]]], axis=0) * SCALE
        wqk = np.ascontiguousarray(np.concatenate([wq, wk], axis=0).T).astype(np_dt)
        bqk = np.concatenate([bq, bk]).astype(f32)
        in_maps.append(
            {
                "xT": np.ascontiguousarray(x[b].T).astype(np_dt),
                "wqk": wqk,
                "bqk": bqk,
                "wv": wvT,
                "bv": bv_,
                "wp": wpT,
                "bp": bp_,
            }
        )
    return in_maps


_NC_CACHE = {}


def kernel(**inputs) -> np.ndarray:
    in_maps = make_in_maps(**inputs)
    if MM_DT not in _NC_CACHE:
        _NC_CACHE[MM_DT] = build_nc(MM_DT)
    nc = _NC_CACHE[MM_DT]
    res = run_bass_kernel_spmd(nc, in_maps, core_ids=list(range(NCORES)))
    out = np.stack([res.results[i]["out"] for i in range(NCORES)], axis=0)
    return out.astype(np.float32)
